# revision 31
# baseline (speedup 1.0000x reference)
"""MoE (top-2, 8 experts) Trainium2 kernel — quarter-shard expert parallelism.

Quarter-shard strategy (load-balanced): each expert's FFN is split into 4 H-shards
(H/4 = 512 columns of W1, matching rows of W2). Cores are organized as two
groups of 4; group g hosts 4 experts, core (g, q) holds H-quarter q of all
4 of its group's experts. All 4 cores of a group process the SAME token
columns (identical x payload) — only the weight slices differ — so the
per-core column layout is group-static and the SPMD program is identical
across cores: segments j=0..3 of fixed length S_j (padded max of the two
groups' expert counts at that position), each segment's chunks using
weight slot j.

Why: per-core PE work drops from 256*max_e(c_e) cycles (expert-parallel,
padded to the heaviest expert) to 64*T, T = sum_j S_j — the expert load
imbalance is split across groups (~110us vs ~116us at this routing).

Host combine: y_e = sum over the 4 quarter-partials (bf16 on the wire),
+ b2, * gate, scatter-add. Rel err ~3.3e-3 (gate: 2e-2).

Measured on trn2 (8 cores, this routing): ~130.2 us HW exec (baseline
378 us), reproducible to ~0.1us: ~11us startup (7us sequencer preamble +
first-chunk DMA, bridged by 36 PE-warmup matmuls), then a ZERO-GAP 114us
single-pass bf16 matmul stream at the N-cycle PE roofline (64*T cycles
@2.4GHz + ~2.5ns/MM issue overhead + occasional HAM cold-start tax), ~5us
drain/teardown. First chunk is sized LONG (408) so chunk 1 starts after
x(1) clears the shared-HBM input ramp.
"""

import contextlib
import ctypes
import itertools
import sys
import types

import numpy as np
import ml_dtypes

B, S, D, H, E, TOPK = 4, 1024, 1024, 2048, 8, 2
N_CORES = 8
P = 128
HQ = H // 4    # 512: H-quarter
KO1 = D // P   # 8  K-tiles in layer 1
M1Q = HQ // P  # 4  M-tiles in layer 1 (per quarter)
KO2Q = HQ // P # 4  K-tiles in layer 2 (per quarter)
M2 = D // P    # 8  M-tiles in layer 2
NSLOT = 4      # experts per core (weight slots)
NT = 512       # max moving-dim chunk width; PSUM fp32 bank is 512 fp32

BF16 = ml_dtypes.bfloat16


def _install_axon_ntff_hook():
    """This image's antenv lacks axon_hooks; inject the ctypes NTFF profiling
    hook so run_bass_kernel_spmd(trace=True) works instead of crashing."""
    try:
        import antenv.axon_hooks  # noqa: F401
        return
    except ImportError:
        pass
    try:
        import antenv
    except ImportError:
        return

    so_path = "/opt/axon/libaxon_pjrt.so"
    try:
        lib = ctypes.CDLL(so_path)
    except OSError:
        lib = None
    hook = None
    if lib is not None and hasattr(lib, "axon_start_nrt_profile"):
        lib.axon_start_nrt_profile.argtypes = [
            ctypes.POINTER(ctypes.c_int64),
            ctypes.c_size_t,
        ]
        lib.axon_start_nrt_profile.restype = ctypes.c_int64
        lib.axon_stop_nrt_profile.argtypes = [ctypes.c_char_p]
        lib.axon_stop_nrt_profile.restype = ctypes.c_int64

        @contextlib.contextmanager
        def hook(output_dir, device_ids):
            import jax

            jax.devices()
            if device_ids:
                ids = (ctypes.c_int64 * len(device_ids))(*device_ids)
                rc = lib.axon_start_nrt_profile(ids, len(device_ids))
            else:
                rc = lib.axon_start_nrt_profile(None, 0)
            if rc != 0:
                raise RuntimeError(f"axon_start_nrt_profile rc={rc}")
            try:
                yield
            finally:
                n = lib.axon_stop_nrt_profile(str(output_dir).encode())
                print(f"profile: {n} file(s) -> {output_dir}", file=sys.stderr)

    mod = types.ModuleType("antenv.axon_hooks")
    state = {"hook": hook}
    mod.set_axon_ntff_profile_hook = lambda h: state.__setitem__("hook", h)
    mod.get_axon_ntff_profile_hook = lambda: state["hook"]
    sys.modules["antenv.axon_hooks"] = mod
    antenv.axon_hooks = mod


def _patch_upload_artifacts():
    """Trace post-processing uploads artifacts to S3; make failures non-fatal."""
    from concourse import bass_utils

    orig = bass_utils.upload_artifacts
    if getattr(orig, "_moe_safe", False):
        return

    def safe_upload(tmpdir):
        try:
            return orig(tmpdir)
        except Exception:
            return f"file://{tmpdir}"

    safe_upload._moe_safe = True
    bass_utils.upload_artifacts = safe_upload


def _split_even(length, first=None):
    """Split `length` into even chunks <= NT, >= ~256 (except tiny totals);
    optionally force the first chunk to `first`."""
    out = []
    if first is not None and 0 < first < length:
        out.append(first)
        length -= first
    n = max(1, -(-length // NT))
    base = (length // n) & ~1
    r = length - base * n
    for i in range(n):
        out.append(base + (2 if i < r // 2 else 0))
    return out


def _layout(segs):
    """[(c0, w, slot)] chunk list for segment lengths `segs` (all even).

    Uniform near-equal chunks per segment: the first chunk is deliberately
    NOT shortened -- the per-core HBM ramp (x0 + slot-0 weights + x1 ~3.5MB
    from DMA-ring start ~8.6us) needs chunk 0 to run ~10us so x(1) lands
    before chunk 1 starts; a short first chunk under-runs the ramp and
    stalls the PE at the chunk boundary."""
    chunks = []
    c0 = 0
    for j, s in enumerate(segs):
        # First chunk slightly LONG (408): chunk 1 then starts ~22us, after
        # x(1) has cleared the shared-HBM ramp (~20.6us).
        for w in _split_even(s, first=408 if j == 0 else None):
            chunks.append((c0, w, j))
            c0 += w
    return chunks


_PROGRAM_CACHE = {}


def _build_program(segs):
    """Per-core bass program: 4 weight slots, segments of token columns, each
    chunk runs the 2-layer quarter-FFN with its segment's slot weights.
    Single-pass bf16 matmuls, fp32 PSUM accumulate.
    """
    import concourse.tile as tile
    from concourse import bacc, mybir

    chunks = _layout(segs)
    T = sum(segs)
    nch = len(chunks)

    nc = bacc.Bacc(None, debug=False)
    bf = mybir.dt.bfloat16
    f32 = mybir.dt.float32
    Alu = mybir.AluOpType

    x_d = nc.dram_tensor("xq", [P * KO1 * T], bf, kind="ExternalInput")
    # slot-major m-blocks: w1 [NSLOT*M1Q, P, KO1*P], w2 [NSLOT*M2, P, KO2Q*P]
    w1_d = nc.dram_tensor("w1q", [NSLOT * M1Q, P, KO1 * P], bf, kind="ExternalInput")
    w2_d = nc.dram_tensor("w2q", [NSLOT * M2, P, KO2Q * P], bf, kind="ExternalInput")
    b1_d = nc.dram_tensor("b1r", [P, NSLOT * M1Q], f32, kind="ExternalInput")
    yT_d = nc.dram_tensor("yT", [D, T], bf, kind="ExternalOutput")

    with tile.TileContext(nc) as tc:
        with (
            tc.tile_pool(name="wpool", bufs=2) as wpool,
            tc.tile_pool(name="xpool", bufs=5) as xpool,
            tc.tile_pool(name="hpool", bufs=2) as hpool,
            tc.tile_pool(name="ypool", bufs=2) as ypool,
            tc.tile_pool(name="bpool", bufs=1) as bpool,
            tc.tile_pool(name="pspool", bufs=7, space="PSUM") as pspool,
            tc.tile_pool(name="wuppool", bufs=1, space="PSUM") as wuppool,
        ):
            # PE warmup: narrow matmuls on a zeroed tile, no DMA deps, result
            # unread; bridges the DMA ramp so the HAM clock gate is released
            # (2.4 GHz) when the real stream starts.
            wup_src = bpool.tile([P, P], bf, tag="wup_src")
            nc.gpsimd.memset(wup_src, 0.0)
            wup_ps = wuppool.tile([P, P], f32, tag="wup_ps")
            # 36 narrow MMs ~= 3.9us of PE busy: enough for the HAM 4096-
            # cycle window to release the clock gate (needs >=3.4us busy,
            # verified on HW), ending ~11.2us -- right at the typical first-
            # chunk DMA arrival. A later DMA leaves a warm-tolerant gap (HAM
            # re-throttles only after >3.4us idle).
            for i in range(32):
                nc.tensor.matmul(
                    wup_ps, wup_src, wup_src,
                    start=(i == 0), stop=(i == 31),
                )

            x_tiles = {}

            def load_x(ci):
                c0, w, _ = chunks[ci]
                off = P * KO1 * c0
                x_t = xpool.tile([P, KO1, w], bf, tag="x")
                src = x_d[off : off + P * KO1 * w].rearrange("(p k) -> p k", p=P)
                if ci == 0:
                    # Ramp-critical: four quarter-DMAs alternating the two
                    # HWDGE rings. Tile-slice dependency tracking lets the
                    # first matmuls (ko 0-1) start once the first quarter +
                    # w1[0] land (~10.3us) instead of waiting for all of x0.
                    for qi in range(4):
                        q = nc.scalar if qi % 2 == 0 else nc.sync
                        ks = slice(2 * qi, 2 * qi + 2)
                        q.dma_start(
                            out=x_t[:, ks],
                            in_=src[:, 2 * qi * w : (2 * qi + 2) * w])
                else:
                    nc.sync.dma_start(out=x_t, in_=src)
                x_tiles[ci] = x_t

            # All 4 slots' weights stay resident (64KB/partition).
            w1 = wpool.tile([P, NSLOT * M1Q, KO1, P], bf, tag="w")
            w2 = wpool.tile([P, NSLOT * M2, KO2Q, P], bf, tag="w")
            # Ramp-critical pair: first w1 m-block alone on Sync, x0 on Scalar.
            nc.sync.dma_start(out=w1[:, 0], in_=w1_d[0])

            load_x(0)  # critical path: first chunk tokens

            b1_sb = bpool.tile([P, NSLOT * M1Q], f32, tag="b1")
            nc.sync.dma_start(out=b1_sb, in_=b1_d[:, :])

            # All remaining input DMAs ride Sync only, pre-issued in DEADLINE
            # order (each x chunk / weight m-block sorted by when the PE
            # first reads it); Scalar is reserved for the y-output drain so
            # neither sequencer's in-order descriptor generation can delay
            # the other's stream. (A 12-block weight burst ahead of x(1)
            # cost a 3.4us PE gap + HAM cold restart before this.)
            t = 0.0
            t_chunk = []
            for _, w, _ in chunks:
                t_chunk.append(t)
                t += 64 * (w / 2.4 + 2.5)
            fc = {}
            for ci2 in range(nch - 1, -1, -1):
                fc[chunks[ci2][2]] = ci2
            # Issue deadline = PE-read time minus that DMA's own transfer +
            # completion lead, so a burst of weight blocks can't queue ahead
            # of a big x chunk it would starve.
            items = []  # (issue_deadline_ns, order, kind, index)
            for ci2 in range(1, nch):
                lead = 2048 * chunks[ci2][1] / 358 + 1500
                items.append((t_chunk[ci2] - lead, 0, "x", ci2))
            w1_lead = 256 * 1024 / 358 + 1500
            w2_lead = KO2Q * P * P * 2 / 358 + 1500
            for s in range(NSLOT):
                t0s = t_chunk[fc[s]]
                w0 = chunks[fc[s]][1]
                l1_dur = 32 * (w0 / 2.4 + 2.5)
                l2_m = KO2Q * (w0 / 2.4 + 2.5)
                for m in range(M1Q):
                    if s == 0 and m == 0:
                        continue
                    items.append((t0s - w1_lead, 1 + m, "w1", s * M1Q + m))
                for m in range(M2):
                    items.append(
                        (t0s + l1_dur + m * l2_m - w2_lead, 1 + m, "w2", s * M2 + m))
            items.sort(key=lambda it: (it[0], it[1]))
            for _, _, kind, i in items:
                if kind == "x":
                    load_x(i)
                elif kind == "w1":
                    nc.sync.dma_start(out=w1[:, i], in_=w1_d[i])
                else:
                    nc.sync.dma_start(out=w2[:, i], in_=w2_d[i])

            # Fused per-chunk L1+L2; the chunk's segment picks the weight slot.
            for ci in range(nch):
                x_t = x_tiles.pop(ci)
                c0, w, slot = chunks[ci]
                csl = slice(c0, c0 + w)
                hT = hpool.tile([P, KO2Q, w], bf, tag="h")
                y_t = ypool.tile([P, M2, w], bf, tag="y")

                # ---- Layer 1: hT = relu(W1q^T @ xT + b1q), bf16 out ----
                for m in range(M1Q):
                    ps = pspool.tile([P, w], f32, tag="ps")
                    for ko in range(KO1):
                        nc.tensor.matmul(
                            ps, w1[:, slot * M1Q + m, ko], x_t[:, ko, :],
                            start=(ko == 0), stop=(ko == KO1 - 1),
                        )
                    nc.vector.tensor_scalar(
                        out=hT[:, m, :], in0=ps,
                        scalar1=b1_sb[:, slot * M1Q + m : slot * M1Q + m + 1],
                        scalar2=0.0,
                        op0=Alu.add, op1=Alu.max,
                    )

                # ---- Layer 2: yT = W2q^T @ hT (quarter partial, bf16 out) ----
                for m in range(M2):
                    ps = pspool.tile([P, w], f32, tag="ps")
                    for ko in range(KO2Q):
                        nc.tensor.matmul(
                            ps, w2[:, slot * M2 + m, ko], hT[:, ko, :],
                            start=(ko == 0), stop=(ko == KO2Q - 1),
                        )
                    # bf16 on the wire halves out-DMA bytes; host sums the
                    # 4 quarter-partials in fp32.
                    nc.vector.tensor_copy(out=y_t[:, m, :], in_=ps)
                    # Combined DMA per 4 m-blocks: descriptor-gen on the
                    # Scalar sequencer is ~0.6us per DMA regardless of size;
                    # 8 per chunk would eat over half the chunk. The last
                    # chunk drains in pairs instead so the final (exposed)
                    # copy+gen+transfer chain is half as long.
                    if ci == nch - 1 and m == M2 - 1:
                        # Exposed final drain: halve the transfer across both
                        # rings (descriptor-gen + transfer run in parallel;
                        # Sync is input-idle by now).
                        wh = (w // 2) & ~1
                        msl = slice(m * P, (m + 1) * P)
                        nc.scalar.dma_start(
                            out=yT_d[msl, c0 : c0 + wh], in_=y_t[:, m, :wh])
                        nc.sync.dma_start(
                            out=yT_d[msl, c0 + wh : c0 + w], in_=y_t[:, m, wh:])
                    else:
                        grp = 1 if ci == nch - 1 else 4
                        if m % grp == grp - 1:
                            m0 = m - grp + 1
                            dst = yT_d[m0 * P : (m + 1) * P, csl].rearrange(
                                "(m p) w -> p m w", p=P)
                            nc.scalar.dma_start(
                                out=dst, in_=y_t[:, m0 : m + 1, :])

    nc.finalize()
    return nc


LAST_EXEC_NS = None
LAST_TRACE = None


def kernel(x, Wg, W1, b1, W2, b2):
    import os

    global LAST_EXEC_NS, LAST_TRACE

    _install_axon_ntff_hook()
    _patch_upload_artifacts()
    from concourse.bass_utils import run_bass_kernel_spmd

    x = np.asarray(x, np.float32)
    Wg = np.asarray(Wg, np.float32)
    W1 = np.asarray(W1, np.float32)
    b1 = np.asarray(b1, np.float32)
    W2 = np.asarray(W2, np.float32)
    b2 = np.asarray(b2, np.float32)

    N = B * S
    xm = np.ascontiguousarray(x.reshape(N, D))

    # --- host routing: identical math to the reference (fp32) ---
    logits = xm @ Wg
    mx = logits.max(-1, keepdims=True)
    ex = np.exp(logits - mx)
    probs = ex / ex.sum(-1, keepdims=True)
    idx = np.argsort(-probs, axis=-1, kind="stable")[:, :TOPK]  # top-2, desc
    p2 = np.take_along_axis(probs, idx, axis=-1)

    toks_per_e = []
    probs_per_e = []
    for e in range(E):
        toks, slots = np.where(idx == e)
        toks_per_e.append(toks)
        probs_per_e.append(p2[toks, slots])
    counts = np.array([len(t) for t in toks_per_e])

    # --- group assignment: 2 groups of 4 experts minimizing sum_j of the
    # positionwise max of (desc-sorted) counts -> minimal padded capacity ---
    best = None
    for g1 in itertools.combinations(range(E), E // 2):
        g2 = tuple(i for i in range(E) if i not in g1)
        a = sorted(g1, key=lambda e: -counts[e])
        b = sorted(g2, key=lambda e: -counts[e])
        segs = tuple(
            (max(counts[ea], counts[eb]) + 1) & ~1 for ea, eb in zip(a, b)
        )
        segs = tuple(max(s, 2) for s in segs)
        if best is None or sum(segs) < best[0]:
            best = (sum(segs), segs, (a, b))
    T, segs, groups = best
    chunks = _layout(segs)
    assert sum(w for _, w, _ in chunks) == T

    def _tile_w1(wq):  # [D, HQ] -> [M1Q, P, KO1*P]
        return np.ascontiguousarray(
            wq.reshape(KO1, P, M1Q, P).transpose(2, 1, 0, 3).reshape(M1Q, P, KO1 * P))

    def _tile_w2(wq):  # [HQ, D] -> [M2, P, KO2Q*P]
        return np.ascontiguousarray(
            wq.reshape(KO2Q, P, M2, P).transpose(2, 1, 0, 3).reshape(M2, P, KO2Q * P))

    def _tile_x(xsT):  # [D, T] -> flat [P*KO1*T], chunk-major [P, KO1, w]
        parts = []
        for c0, w, _ in chunks:
            blk = xsT[:, c0:c0 + w].reshape(KO1, P, w).transpose(1, 0, 2)
            parts.append(np.ascontiguousarray(blk).reshape(-1))
        return np.concatenate(parts)

    # --- per-group token payload (shared by the 4 cores of the group) ---
    xmT = np.ascontiguousarray(xm.T)  # [D, N]
    xq_per_group = []
    for g in range(2):
        xsT = np.zeros((D, T), np.float32)
        for j, e in enumerate(groups[g]):
            c0 = sum(segs[:j])
            toks = toks_per_e[e]
            xsT[:, c0 : c0 + len(toks)] = xmT[:, toks]
        xq_per_group.append(_tile_x(xsT.astype(BF16)))

    # --- per-core inputs: core = g*4 + q holds H-quarter q of group g ---
    in_maps = []
    for g in range(2):
        for q in range(4):
            hsl = slice(q * HQ, (q + 1) * HQ)
            w1q = np.concatenate(
                [_tile_w1(W1[e][:, hsl].astype(BF16)) for e in groups[g]])
            w2q = np.concatenate(
                [_tile_w2(W2[e][hsl, :].astype(BF16)) for e in groups[g]])
            b1r = np.concatenate(
                [np.ascontiguousarray(b1[e][hsl].reshape(M1Q, P).T)
                 for e in groups[g]], axis=1)  # [128, 16]
            in_maps.append({
                "xq": xq_per_group[g],
                "w1q": w1q, "w2q": w2q, "b1r": b1r,
            })

    if segs not in _PROGRAM_CACHE:
        _PROGRAM_CACHE[segs] = _build_program(segs)
    nc = _PROGRAM_CACHE[segs]

    trace = os.environ.get("BASS_MOE_TRACE", "").strip() in ("1", "true", "yes")
    kw = {}
    if trace:
        kw["trace"] = True
        tdir = os.environ.get("BASS_MOE_TRACE_DIR")
        if tdir:
            kw["tmpdir"] = tdir
    res = run_bass_kernel_spmd(nc, in_maps, core_ids=list(range(N_CORES)), **kw)
    LAST_EXEC_NS = res.exec_time_ns
    LAST_TRACE = res.instructions_and_trace[1] if res.instructions_and_trace else None

    # --- host combine: sum quarter-partials, bias2 + gates + scatter-add ---
    out = np.zeros((N, D), np.float32)
    for g in range(2):
        parts = [np.asarray(res.results[g * 4 + q]["yT"], np.float32)
                 for q in range(4)]
        ysum = parts[0] + parts[1] + parts[2] + parts[3]  # [D, T] fp32
        for j, e in enumerate(groups[g]):
            toks = toks_per_e[e]
            if len(toks) == 0:
                continue
            c0 = sum(segs[:j])
            y = ysum[:, c0 : c0 + len(toks)].T  # [c_e, D]
            out[toks] += (y + b2[e]) * probs_per_e[e][:, None]
    return out.reshape(B, S, D)


# revision 32
# speedup vs baseline: 1.0042x; 1.0042x over previous
"""MoE (top-2, 8 experts) Trainium2 kernel — quarter-shard expert parallelism.

Quarter-shard strategy (load-balanced): each expert's FFN is split into 4 H-shards
(H/4 = 512 columns of W1, matching rows of W2). Cores are organized as two
groups of 4; group g hosts 4 experts, core (g, q) holds H-quarter q of all
4 of its group's experts. All 4 cores of a group process the SAME token
columns (identical x payload) — only the weight slices differ — so the
per-core column layout is group-static and the SPMD program is identical
across cores: segments j=0..3 of fixed length S_j (padded max of the two
groups' expert counts at that position), each segment's chunks using
weight slot j.

Why: per-core PE work drops from 256*max_e(c_e) cycles (expert-parallel,
padded to the heaviest expert) to 64*T, T = sum_j S_j — the expert load
imbalance is split across groups (~110us vs ~116us at this routing).

Host combine: y_e = sum over the 4 quarter-partials (bf16 on the wire),
+ b2, * gate, scatter-add. Rel err ~3.3e-3 (gate: 2e-2).

Measured on trn2 (8 cores, this routing): ~130.2 us HW exec (baseline
378 us), reproducible to ~0.1us: ~11us startup (7us sequencer preamble +
first-chunk DMA, bridged by 36 PE-warmup matmuls), then a ZERO-GAP 114us
single-pass bf16 matmul stream at the N-cycle PE roofline (64*T cycles
@2.4GHz + ~2.5ns/MM issue overhead + occasional HAM cold-start tax), ~5us
drain/teardown. First chunk is sized LONG (408) so chunk 1 starts after
x(1) clears the shared-HBM input ramp.
"""

import contextlib
import ctypes
import itertools
import sys
import types

import numpy as np
import ml_dtypes

B, S, D, H, E, TOPK = 4, 1024, 1024, 2048, 8, 2
N_CORES = 8
P = 128
HQ = H // 4    # 512: H-quarter
KO1 = D // P   # 8  K-tiles in layer 1
M1Q = HQ // P  # 4  M-tiles in layer 1 (per quarter)
KO2Q = HQ // P # 4  K-tiles in layer 2 (per quarter)
M2 = D // P    # 8  M-tiles in layer 2
NSLOT = 4      # experts per core (weight slots)
NT = 512       # max moving-dim chunk width; PSUM fp32 bank is 512 fp32

BF16 = ml_dtypes.bfloat16


def _install_axon_ntff_hook():
    """This image's antenv lacks axon_hooks; inject the ctypes NTFF profiling
    hook so run_bass_kernel_spmd(trace=True) works instead of crashing."""
    try:
        import antenv.axon_hooks  # noqa: F401
        return
    except ImportError:
        pass
    try:
        import antenv
    except ImportError:
        return

    so_path = "/opt/axon/libaxon_pjrt.so"
    try:
        lib = ctypes.CDLL(so_path)
    except OSError:
        lib = None
    hook = None
    if lib is not None and hasattr(lib, "axon_start_nrt_profile"):
        lib.axon_start_nrt_profile.argtypes = [
            ctypes.POINTER(ctypes.c_int64),
            ctypes.c_size_t,
        ]
        lib.axon_start_nrt_profile.restype = ctypes.c_int64
        lib.axon_stop_nrt_profile.argtypes = [ctypes.c_char_p]
        lib.axon_stop_nrt_profile.restype = ctypes.c_int64

        @contextlib.contextmanager
        def hook(output_dir, device_ids):
            import jax

            jax.devices()
            if device_ids:
                ids = (ctypes.c_int64 * len(device_ids))(*device_ids)
                rc = lib.axon_start_nrt_profile(ids, len(device_ids))
            else:
                rc = lib.axon_start_nrt_profile(None, 0)
            if rc != 0:
                raise RuntimeError(f"axon_start_nrt_profile rc={rc}")
            try:
                yield
            finally:
                n = lib.axon_stop_nrt_profile(str(output_dir).encode())
                print(f"profile: {n} file(s) -> {output_dir}", file=sys.stderr)

    mod = types.ModuleType("antenv.axon_hooks")
    state = {"hook": hook}
    mod.set_axon_ntff_profile_hook = lambda h: state.__setitem__("hook", h)
    mod.get_axon_ntff_profile_hook = lambda: state["hook"]
    sys.modules["antenv.axon_hooks"] = mod
    antenv.axon_hooks = mod


def _patch_upload_artifacts():
    """Trace post-processing uploads artifacts to S3; make failures non-fatal."""
    from concourse import bass_utils

    orig = bass_utils.upload_artifacts
    if getattr(orig, "_moe_safe", False):
        return

    def safe_upload(tmpdir):
        try:
            return orig(tmpdir)
        except Exception:
            return f"file://{tmpdir}"

    safe_upload._moe_safe = True
    bass_utils.upload_artifacts = safe_upload


def _split_even(length, first=None):
    """Split `length` into even chunks <= NT, >= ~256 (except tiny totals);
    optionally force the first chunk to `first`."""
    out = []
    if first is not None and 0 < first < length:
        out.append(first)
        length -= first
    n = max(1, -(-length // NT))
    base = (length // n) & ~1
    r = length - base * n
    for i in range(n):
        out.append(base + (2 if i < r // 2 else 0))
    return out


def _layout(segs):
    """[(c0, w, slot)] chunk list for segment lengths `segs` (all even).

    Uniform near-equal chunks per segment: the first chunk is deliberately
    NOT shortened -- the per-core HBM ramp (x0 + slot-0 weights + x1 ~3.5MB
    from DMA-ring start ~8.6us) needs chunk 0 to run ~10us so x(1) lands
    before chunk 1 starts; a short first chunk under-runs the ramp and
    stalls the PE at the chunk boundary."""
    chunks = []
    c0 = 0
    for j, s in enumerate(segs):
        # First chunk slightly LONG (408): chunk 1 then starts ~22us, after
        # x(1) has cleared the shared-HBM ramp (~20.6us).
        for w in _split_even(s, first=408 if j == 0 else None):
            chunks.append((c0, w, j))
            c0 += w
    return chunks


_PROGRAM_CACHE = {}


def _build_program(segs):
    """Per-core bass program: 4 weight slots, segments of token columns, each
    chunk runs the 2-layer quarter-FFN with its segment's slot weights.
    Single-pass bf16 matmuls, fp32 PSUM accumulate.
    """
    import concourse.tile as tile
    from concourse import bacc, mybir

    chunks = _layout(segs)
    T = sum(segs)
    nch = len(chunks)

    nc = bacc.Bacc(None, debug=False)
    bf = mybir.dt.bfloat16
    f32 = mybir.dt.float32
    Alu = mybir.AluOpType

    x_d = nc.dram_tensor("xq", [P * KO1 * T], bf, kind="ExternalInput")
    # slot-major m-blocks: w1 [NSLOT*M1Q, P, KO1*P], w2 [NSLOT*M2, P, KO2Q*P]
    w1_d = nc.dram_tensor("w1q", [NSLOT * M1Q, P, KO1 * P], bf, kind="ExternalInput")
    w2_d = nc.dram_tensor("w2q", [NSLOT * M2, P, KO2Q * P], bf, kind="ExternalInput")
    b1_d = nc.dram_tensor("b1r", [P, NSLOT * M1Q], f32, kind="ExternalInput")
    yT_d = nc.dram_tensor("yT", [D, T], bf, kind="ExternalOutput")

    with tile.TileContext(nc) as tc:
        with (
            tc.tile_pool(name="wpool", bufs=2) as wpool,
            tc.tile_pool(name="xpool", bufs=5) as xpool,
            tc.tile_pool(name="hpool", bufs=2) as hpool,
            tc.tile_pool(name="ypool", bufs=2) as ypool,
            tc.tile_pool(name="bpool", bufs=1) as bpool,
            tc.tile_pool(name="pspool", bufs=7, space="PSUM") as pspool,
            tc.tile_pool(name="wuppool", bufs=1, space="PSUM") as wuppool,
        ):
            # PE warmup: narrow matmuls on a zeroed tile, no DMA deps, result
            # unread; bridges the DMA ramp so the HAM clock gate is released
            # (2.4 GHz) when the real stream starts.
            wup_src = bpool.tile([P, P], bf, tag="wup_src")
            nc.gpsimd.memset(wup_src, 0.0)
            wup_ps = wuppool.tile([P, P], f32, tag="wup_ps")
            # 36 narrow MMs ~= 3.9us of PE busy: enough for the HAM 4096-
            # cycle window to release the clock gate (needs >=3.4us busy,
            # verified on HW), ending ~11.2us -- right at the typical first-
            # chunk DMA arrival. A later DMA leaves a warm-tolerant gap (HAM
            # re-throttles only after >3.4us idle).
            for i in range(36):
                nc.tensor.matmul(
                    wup_ps, wup_src, wup_src,
                    start=(i == 0), stop=(i == 35),
                )

            x_tiles = {}

            def load_x(ci):
                c0, w, _ = chunks[ci]
                off = P * KO1 * c0
                x_t = xpool.tile([P, KO1, w], bf, tag="x")
                src = x_d[off : off + P * KO1 * w].rearrange("(p k) -> p k", p=P)
                if ci == 0:
                    # Ramp-critical: halves ride both HWDGE rings in parallel
                    # (scalar first half; sync second half, after w1[0]).
                    hw = (KO1 // 2) * w
                    nc.scalar.dma_start(out=x_t[:, : KO1 // 2], in_=src[:, :hw])
                    nc.sync.dma_start(out=x_t[:, KO1 // 2 :], in_=src[:, hw:])
                else:
                    nc.sync.dma_start(out=x_t, in_=src)
                x_tiles[ci] = x_t

            # All 4 slots' weights stay resident (64KB/partition).
            w1 = wpool.tile([P, NSLOT * M1Q, KO1, P], bf, tag="w")
            w2 = wpool.tile([P, NSLOT * M2, KO2Q, P], bf, tag="w")
            # Ramp-critical pair: first w1 m-block alone on Sync, x0 on Scalar.
            nc.sync.dma_start(out=w1[:, 0], in_=w1_d[0])

            load_x(0)  # critical path: first chunk tokens

            b1_sb = bpool.tile([P, NSLOT * M1Q], f32, tag="b1")
            nc.sync.dma_start(out=b1_sb, in_=b1_d[:, :])

            # All remaining input DMAs ride Sync only, pre-issued in DEADLINE
            # order (each x chunk / weight m-block sorted by when the PE
            # first reads it); Scalar is reserved for the y-output drain so
            # neither sequencer's in-order descriptor generation can delay
            # the other's stream. (A 12-block weight burst ahead of x(1)
            # cost a 3.4us PE gap + HAM cold restart before this.)
            t = 0.0
            t_chunk = []
            for _, w, _ in chunks:
                t_chunk.append(t)
                t += 64 * (w / 2.4 + 2.5)
            fc = {}
            for ci2 in range(nch - 1, -1, -1):
                fc[chunks[ci2][2]] = ci2
            # Issue deadline = PE-read time minus that DMA's own transfer +
            # completion lead, so a burst of weight blocks can't queue ahead
            # of a big x chunk it would starve.
            items = []  # (issue_deadline_ns, order, kind, index)
            for ci2 in range(1, nch):
                lead = 2048 * chunks[ci2][1] / 358 + 1500
                items.append((t_chunk[ci2] - lead, 0, "x", ci2))
            w1_lead = 256 * 1024 / 358 + 1500
            w2_lead = KO2Q * P * P * 2 / 358 + 1500
            for s in range(NSLOT):
                t0s = t_chunk[fc[s]]
                w0 = chunks[fc[s]][1]
                l1_dur = 32 * (w0 / 2.4 + 2.5)
                l2_m = KO2Q * (w0 / 2.4 + 2.5)
                for m in range(M1Q):
                    if s == 0 and m == 0:
                        continue
                    items.append((t0s - w1_lead, 1 + m, "w1", s * M1Q + m))
                for m in range(M2):
                    items.append(
                        (t0s + l1_dur + m * l2_m - w2_lead, 1 + m, "w2", s * M2 + m))
            items.sort(key=lambda it: (it[0], it[1]))
            for _, _, kind, i in items:
                if kind == "x":
                    load_x(i)
                elif kind == "w1":
                    nc.sync.dma_start(out=w1[:, i], in_=w1_d[i])
                else:
                    nc.sync.dma_start(out=w2[:, i], in_=w2_d[i])

            # Fused per-chunk L1+L2; the chunk's segment picks the weight slot.
            for ci in range(nch):
                x_t = x_tiles.pop(ci)
                c0, w, slot = chunks[ci]
                csl = slice(c0, c0 + w)
                hT = hpool.tile([P, KO2Q, w], bf, tag="h")
                y_t = ypool.tile([P, M2, w], bf, tag="y")

                # ---- Layer 1: hT = relu(W1q^T @ xT + b1q), bf16 out ----
                for m in range(M1Q):
                    ps = pspool.tile([P, w], f32, tag="ps")
                    for ko in range(KO1):
                        nc.tensor.matmul(
                            ps, w1[:, slot * M1Q + m, ko], x_t[:, ko, :],
                            start=(ko == 0), stop=(ko == KO1 - 1),
                        )
                    nc.vector.tensor_scalar(
                        out=hT[:, m, :], in0=ps,
                        scalar1=b1_sb[:, slot * M1Q + m : slot * M1Q + m + 1],
                        scalar2=0.0,
                        op0=Alu.add, op1=Alu.max,
                    )

                # ---- Layer 2: yT = W2q^T @ hT (quarter partial, bf16 out) ----
                for m in range(M2):
                    ps = pspool.tile([P, w], f32, tag="ps")
                    for ko in range(KO2Q):
                        nc.tensor.matmul(
                            ps, w2[:, slot * M2 + m, ko], hT[:, ko, :],
                            start=(ko == 0), stop=(ko == KO2Q - 1),
                        )
                    # bf16 on the wire halves out-DMA bytes; host sums the
                    # 4 quarter-partials in fp32.
                    nc.vector.tensor_copy(out=y_t[:, m, :], in_=ps)
                    # Combined DMA per 4 m-blocks: descriptor-gen on the
                    # Scalar sequencer is ~0.6us per DMA regardless of size;
                    # 8 per chunk would eat over half the chunk. The last
                    # chunk drains in pairs instead so the final (exposed)
                    # copy+gen+transfer chain is half as long.
                    grp = 1 if ci == nch - 1 else 4
                    if m % grp == grp - 1:
                        m0 = m - grp + 1
                        dst = yT_d[m0 * P : (m + 1) * P, csl].rearrange(
                            "(m p) w -> p m w", p=P)
                        nc.scalar.dma_start(out=dst, in_=y_t[:, m0 : m + 1, :])

    nc.finalize()
    return nc


LAST_EXEC_NS = None
LAST_TRACE = None


def kernel(x, Wg, W1, b1, W2, b2):
    import os

    global LAST_EXEC_NS, LAST_TRACE

    _install_axon_ntff_hook()
    _patch_upload_artifacts()
    from concourse.bass_utils import run_bass_kernel_spmd

    x = np.asarray(x, np.float32)
    Wg = np.asarray(Wg, np.float32)
    W1 = np.asarray(W1, np.float32)
    b1 = np.asarray(b1, np.float32)
    W2 = np.asarray(W2, np.float32)
    b2 = np.asarray(b2, np.float32)

    N = B * S
    xm = np.ascontiguousarray(x.reshape(N, D))

    # --- host routing: identical math to the reference (fp32) ---
    logits = xm @ Wg
    mx = logits.max(-1, keepdims=True)
    ex = np.exp(logits - mx)
    probs = ex / ex.sum(-1, keepdims=True)
    idx = np.argsort(-probs, axis=-1, kind="stable")[:, :TOPK]  # top-2, desc
    p2 = np.take_along_axis(probs, idx, axis=-1)

    toks_per_e = []
    probs_per_e = []
    for e in range(E):
        toks, slots = np.where(idx == e)
        toks_per_e.append(toks)
        probs_per_e.append(p2[toks, slots])
    counts = np.array([len(t) for t in toks_per_e])

    # --- group assignment: 2 groups of 4 experts minimizing sum_j of the
    # positionwise max of (desc-sorted) counts -> minimal padded capacity ---
    best = None
    for g1 in itertools.combinations(range(E), E // 2):
        g2 = tuple(i for i in range(E) if i not in g1)
        a = sorted(g1, key=lambda e: -counts[e])
        b = sorted(g2, key=lambda e: -counts[e])
        segs = tuple(
            (max(counts[ea], counts[eb]) + 1) & ~1 for ea, eb in zip(a, b)
        )
        segs = tuple(max(s, 2) for s in segs)
        if best is None or sum(segs) < best[0]:
            best = (sum(segs), segs, (a, b))
    T, segs, groups = best
    chunks = _layout(segs)
    assert sum(w for _, w, _ in chunks) == T

    def _tile_w1(wq):  # [D, HQ] -> [M1Q, P, KO1*P]
        return np.ascontiguousarray(
            wq.reshape(KO1, P, M1Q, P).transpose(2, 1, 0, 3).reshape(M1Q, P, KO1 * P))

    def _tile_w2(wq):  # [HQ, D] -> [M2, P, KO2Q*P]
        return np.ascontiguousarray(
            wq.reshape(KO2Q, P, M2, P).transpose(2, 1, 0, 3).reshape(M2, P, KO2Q * P))

    def _tile_x(xsT):  # [D, T] -> flat [P*KO1*T], chunk-major [P, KO1, w]
        parts = []
        for c0, w, _ in chunks:
            blk = xsT[:, c0:c0 + w].reshape(KO1, P, w).transpose(1, 0, 2)
            parts.append(np.ascontiguousarray(blk).reshape(-1))
        return np.concatenate(parts)

    # --- per-group token payload (shared by the 4 cores of the group) ---
    xmT = np.ascontiguousarray(xm.T)  # [D, N]
    xq_per_group = []
    for g in range(2):
        xsT = np.zeros((D, T), np.float32)
        for j, e in enumerate(groups[g]):
            c0 = sum(segs[:j])
            toks = toks_per_e[e]
            xsT[:, c0 : c0 + len(toks)] = xmT[:, toks]
        xq_per_group.append(_tile_x(xsT.astype(BF16)))

    # --- per-core inputs: core = g*4 + q holds H-quarter q of group g ---
    in_maps = []
    for g in range(2):
        for q in range(4):
            hsl = slice(q * HQ, (q + 1) * HQ)
            w1q = np.concatenate(
                [_tile_w1(W1[e][:, hsl].astype(BF16)) for e in groups[g]])
            w2q = np.concatenate(
                [_tile_w2(W2[e][hsl, :].astype(BF16)) for e in groups[g]])
            b1r = np.concatenate(
                [np.ascontiguousarray(b1[e][hsl].reshape(M1Q, P).T)
                 for e in groups[g]], axis=1)  # [128, 16]
            in_maps.append({
                "xq": xq_per_group[g],
                "w1q": w1q, "w2q": w2q, "b1r": b1r,
            })

    if segs not in _PROGRAM_CACHE:
        _PROGRAM_CACHE[segs] = _build_program(segs)
    nc = _PROGRAM_CACHE[segs]

    trace = os.environ.get("BASS_MOE_TRACE", "").strip() in ("1", "true", "yes")
    kw = {}
    if trace:
        kw["trace"] = True
        tdir = os.environ.get("BASS_MOE_TRACE_DIR")
        if tdir:
            kw["tmpdir"] = tdir
    res = run_bass_kernel_spmd(nc, in_maps, core_ids=list(range(N_CORES)), **kw)
    LAST_EXEC_NS = res.exec_time_ns
    LAST_TRACE = res.instructions_and_trace[1] if res.instructions_and_trace else None

    # --- host combine: sum quarter-partials, bias2 + gates + scatter-add ---
    out = np.zeros((N, D), np.float32)
    for g in range(2):
        parts = [np.asarray(res.results[g * 4 + q]["yT"], np.float32)
                 for q in range(4)]
        ysum = parts[0] + parts[1] + parts[2] + parts[3]  # [D, T] fp32
        for j, e in enumerate(groups[g]):
            toks = toks_per_e[e]
            if len(toks) == 0:
                continue
            c0 = sum(segs[:j])
            y = ysum[:, c0 : c0 + len(toks)].T  # [c_e, D]
            out[toks] += (y + b2[e]) * probs_per_e[e][:, None]
    return out.reshape(B, S, D)
